# revision 9
# baseline (speedup 1.0000x reference)
"""Causal GQA self-attention (B=2, S=2048, D=2048, 16 Q heads / 4 KV heads)
on 8 Trainium2 NeuronCores.

Sharding: core i handles (batch b = i // 4, kv-head group g = i % 4) — one
batch element and 4 consecutive query heads + their shared KV head. Each
core computes its heads' attention and a partial output projection
(columns g*512:(g+1)*512 of wo contracted); the host sums the 4 partials
per batch (fp32), descales by 2^-10, and adds the output bias.

Precision/performance plan:
- All projection matmuls (Q/K/V and the output projection) run as fp8
  e4m3 DoubleRow matmuls (2 contraction k-tiles per instruction, 0.5
  PE cycles per output column) with a 3-term error-corrected operand
  split: x@w ~= x_hi@w_hi + x_lo@w_hi + x_hi@w_lo, where t_hi = fp8(t),
  t_lo = fp8(t - t_hi). Per-tensor power-of-2 scales (folded into the
  psum-draining activation's scale, and into the host descale for the
  output) keep the lo residuals above e4m3's subnormal floor. Net
  precision is BETTER than a bf16 kernel (~0.25% max-rel).
- Attention runs in fp16: Q^T/K^T from the psum via act (descale+bias),
  scores = K^T.T @ Q^T in fp16 (causally trimmed at 128-column
  granularity), A = exp(s - ln64) in fp16 (the shift cancels in the
  softmax normalization; it keeps fp16 row sums < 1e3), A@V in fp16.
- Softmax denominator: fp16 sumacc chain on DVE (2x packed mode), then
  a ones(1/32)-matrix matmul broadcasts the partition-sum into PSUM;
  DVE reciprocal gives rb = 32/denom, so t = y_ps * rb = 32*yn which is
  split to fp8 hi/lo as the out-projection's moving operand.
- On-chip layout transposed (feature-on-partition) throughout, as in
  the bf16 predecessor; V natural tiles via DMA transpose.
"""

import math

import numpy as np
import ml_dtypes

import concourse.bass as bass
import concourse.mybir as mybir
import concourse.tile as tile
from concourse.bass_utils import run_bass_kernel_spmd

BF16 = mybir.dt.bfloat16
F16 = mybir.dt.float16
F32 = mybir.dt.float32
FP8 = mybir.dt.float8e4

B = 2
S = 2048
D = 2048
N_HEAD = 16
HEAD_DIM = 128
N_KV = 4
GQ = N_HEAD // N_KV          # heads per group = 4
EG = GQ * HEAD_DIM           # embed dims per group = 512
SC = 512                     # seq chunk (moving-dim width)
NSC = S // SC                # 4 seq chunks
NT = D // 128                # 16 d-tiles
NST = S // 128               # 16 seq k-tiles
ACT = mybir.ActivationFunctionType
DR = mybir.MatmulPerfMode.DoubleRow

# power-of-2 fp8 scales (host-side); descales fold into act scale / host
SWQ = 512.0   # also absorbs the 1/sqrt(128) attention scale making wq tiny
SWK = 32.0
SWV = 32.0
SWO = 32.0
SYN = 32.0    # yn staged as 32*yn (via ones = 1/32)
OUT_DESCALE = 1.0 / (SYN * SWO)   # host-side
EXP_SHIFT = -math.log(64.0)

_CACHE = {}


def _build_nc():
    nc = bass.Bass("TRN2", target_bir_lowering=False)

    # Inputs host-prearranged to exact SBUF layout [128, X]. DMA order is
    # chosen so the K projection can start ~3us in (HWDGE descriptor
    # processing is serial; transfers serialize on the DMA-engine pool).
    xh = [nc.declare_dram_parameter(f"xh{c}", [128, NT * SC], FP8,
                                    isOutput=False) for c in range(NSC)]
    xl = [nc.declare_dram_parameter(f"xl{c}", [128, NT * SC], FP8,
                                    isOutput=False) for c in range(NSC)]
    wqA = nc.declare_dram_parameter("wqA", [128, NT * EG], FP8, isOutput=False)
    wqB = nc.declare_dram_parameter("wqB", [128, NT * EG], FP8, isOutput=False)
    wkv4 = [nc.declare_dram_parameter(n, [128, NT * HEAD_DIM], FP8,
                                      isOutput=False)
            for n in ("wkhi", "wklo", "wvhi", "wvlo")]
    wo = nc.declare_dram_parameter("wo", [128, 2 * GQ * D], FP8, isOutput=False)
    biases = nc.declare_dram_parameter("biases", [128, 7], F32, isOutput=False)
    masks = nc.declare_dram_parameter("masks", [128, 128], F16, isOutput=False)
    outT = nc.declare_dram_parameter("outT", [D, S], BF16, isOutput=True)

    with tile.TileContext(nc) as tc:
        with (
            tc.tile_pool(name="persist", bufs=1) as pp,
            tc.tile_pool(name="rot", bufs=1) as rp,
            tc.tile_pool(name="ps", bufs=1, space="PSUM") as ps,
        ):
            # ---- constants (no DMA deps) ----
            ones_sq = pp.tile([128, 128], F16, name="ones_sq")
            nc.vector.memset(ones_sq[:], 1.0 / SYN)

            # ---- batched loads, priority order ----
            wkv_sb = [None] * 4
            x_sb = [[None] * NSC, [None] * NSC]   # [hi/lo][chunk]
            x0h_parts, x0l_parts = [], []

            def load(name, shape, src):
                t = pp.tile([128, shape], FP8, name=name)
                nc.sync.dma_start(t[:], src[:] if not isinstance(src, tuple)
                                  else src[0][:, src[1]:src[2]])
                return t

            wkv_sb[0] = load("wkhi_sb", NT * HEAD_DIM, wkv4[0])
            q4 = NT * SC // 4
            for q in range(4):
                x0h_parts.append(load(f"x0h{q}", q4, (xh[0], q * q4,
                                                      (q + 1) * q4)))
            wkv_sb[1] = load("wklo_sb", NT * HEAD_DIM, wkv4[1])
            for q in range(4):
                x0l_parts.append(load(f"x0l{q}", q4, (xl[0], q * q4,
                                                      (q + 1) * q4)))
            wkv_sb[2] = load("wvhi_sb", NT * HEAD_DIM, wkv4[2])
            wkv_sb[3] = load("wvlo_sb", NT * HEAD_DIM, wkv4[3])
            wqA_sb = load("wqA_sb", NT * EG, wqA)
            b_sb = pp.tile([128, 7], F32, name="b_sb")
            nc.sync.dma_start(b_sb[:], biases[:])
            m_sb = pp.tile([128, 128], F16, name="m_sb")
            nc.sync.dma_start(m_sb[:], masks[:])
            wqB_sb = load("wqB_sb", NT * EG, wqB)
            x_sb[0][1] = load("xh_sb1", NT * SC, xh[1])
            x_sb[1][1] = load("xl_sb1", NT * SC, xl[1])
            wo_sb = load("wo_sb", 2 * GQ * D, wo)
            for c in range(2, NSC):
                x_sb[0][c] = load(f"xh_sb{c}", NT * SC, xh[c])
                x_sb[1][c] = load(f"xl_sb{c}", NT * SC, xl[c])

            def x_pair(lo, c, t2):
                """Moving pair [128, 2, SC] for dt pair (2*t2, 2*t2+1)."""
                if c == 0:
                    parts = x0l_parts if lo else x0h_parts
                    t = parts[t2 // 2]          # 4 dts per quarter = 2 pairs
                    v = t[:].rearrange("p (n j) -> p n j", j=SC)
                    j = (t2 % 2) * 2
                    return v[:, j:j + 2, :]
                v = x_sb[lo][c][:].rearrange("p (n j) -> p n j", j=SC)
                return v[:, 2 * t2:2 * t2 + 2, :]

            def wkv_pair(which, t2):
                v = wkv_sb[which][:].rearrange("p (n j) -> p n j", j=HEAD_DIM)
                return v[:, 2 * t2:2 * t2 + 2, :]

            def wq_pair(lo, h, t2):
                t = wqA_sb if h < 2 else wqB_sb
                v = t[:].rearrange("p (n k j) -> p n k j", k=2, j=2 * HEAD_DIM)
                # layout per dt: [h0h1_hi | h0h1_lo] (k=hi/lo, j=2 heads)
                hh = h % 2
                return v[:, 2 * t2:2 * t2 + 2, lo,
                         hh * HEAD_DIM:(hh + 1) * HEAD_DIM]

            def wo_pair(lo, hp, ft):
                v = wo_sb[:].rearrange("p (k n j) -> p k n j", k=2, j=D)
                return v[:, lo, 2 * hp:2 * hp + 2, ft * 128:(ft + 1) * 128]

            KT_sb, VT_sb, V_sb = [None] * NSC, [None] * NSC, [None] * NST
            QT = {}
            YnH = [[None] * 2 for _ in range(NSC)]   # [c][hp] fp8 pair tiles
            YnL = [[None] * 2 for _ in range(NSC)]

            def proj_mm(psum, w_pair_fn, c):
                """24 DoubleRow matmuls: main, w-corr, x-corr terms."""
                n2 = NT // 2
                terms = ([(0, 0)] * n2) + ([(1, 0)] * n2) + ([(0, 1)] * n2)
                for i, (wlo, xlo) in enumerate(terms):
                    t2 = i % n2
                    nc.tensor.matmul(
                        psum[:], w_pair_fn(wlo, t2), x_pair(xlo, c, t2),
                        start=(i == 0), stop=(i == 3 * n2 - 1), perf_mode=DR)

            def proj_groups(c):
                """Filler closures (est_PE_ns, fn) producing chunk c's
                K^T/V^T/Q^T (+ V natural tiles)."""
                def kv(which, bias_col, out_list, label, sc, transpose):
                    def g():
                        psum = ps.tile([128, SC], F32, name=f"{label}ps{c}",
                                       tag="mm", bufs=5)
                        proj_mm(psum,
                                lambda lo, t2: wkv_pair(which + lo, t2), c)
                        o = pp.tile([128, SC], F16, name=f"{label}{c}")
                        nc.scalar.activation(
                            o[:], psum[:], ACT.Identity,
                            bias=b_sb[:, bias_col:bias_col + 1], scale=sc)
                        out_list[c] = o
                        if transpose:
                            for j in range(4):
                                st = 4 * c + j
                                v = pp.tile([128, 128], F16, name=f"V{st}")
                                nc.sync.dma_start(
                                    v[:], o[:, j * 128:(j + 1) * 128],
                                    transpose=True)
                                V_sb[st] = v
                    return g

                def qg(h):
                    def g():
                        psum = ps.tile([128, SC], F32, name=f"QTps{h}_{c}",
                                       tag="mm", bufs=5)
                        proj_mm(psum, lambda lo, t2: wq_pair(lo, h, t2), c)
                        q = rp.tile([128, SC], F16, name=f"QT{h}_{c}",
                                    tag=f"qt{h}", bufs=2)
                        nc.scalar.activation(q[:], psum[:], ACT.Identity,
                                             bias=b_sb[:, h:h + 1],
                                             scale=1.0 / SWQ)
                        QT[(h, c)] = q
                    return g

                return [
                    (2560, kv(0, 4, KT_sb, "KT", 1.0 / SWK, False)),
                    (2560, kv(2, 5, VT_sb, "VT", 1.0 / SWV, True)),
                    (2560, qg(0)), (2560, qg(1)),
                    (2560, qg(2)), (2560, qg(3)),
                ]

            def outproj_groups(c):
                """Filler closures for chunk c's output projection; the
                o_stage DMA for a range is emitted right after its last ft."""
                o_stage = rp.tile([128, NT * SC], BF16, name=f"ostage{c}",
                                  tag="ostage", bufs=2)
                nsplit = 8 if c == NSC - 1 else 2
                step = NT // nsplit
                outT_v = outT.rearrange("(n p) s -> p n s", p=128)[
                    :, :, c * SC:(c + 1) * SC]
                o_stage_v = o_stage[:].rearrange("p (n j) -> p n j", j=SC)

                def ft_group(ft):
                    def g():
                        o_ps = ps.tile([128, SC], F32, name=f"ops{c}_{ft}",
                                       tag="mm", bufs=5)
                        i = 0
                        for wlo, yn in ((0, YnH), (1, YnH), (0, YnL)):
                            for hp in range(2):
                                nc.tensor.matmul(
                                    o_ps[:], wo_pair(wlo, hp, ft),
                                    yn[c][hp][:, 0:2, :],
                                    start=(i == 0), stop=(i == 5),
                                    perf_mode=DR)
                                i += 1
                        dst = o_stage[:, ft * SC:(ft + 1) * SC]
                        # engine split: keep chunk-3-window act exp-only
                        if c == 2 or (c < 2 and ft % 2):
                            nc.vector.tensor_copy(dst, o_ps[:])
                        else:
                            nc.scalar.activation(dst, o_ps[:], ACT.Copy)
                        if (ft + 1) % step == 0:
                            qq = ft // step
                            nc.gpsimd.dma_start(
                                outT_v[:, qq * step:(qq + 1) * step],
                                o_stage_v[:, qq * step:(qq + 1) * step])
                    return g

                return [(640, ft_group(ft)) for ft in range(NT)]

            def attention(c, filler, interleave=True,
                          flush_pair=False):
                n_kt = 4 * (c + 1)
                y_ps_l, sumacc_l = {}, {}
                late = c >= 2     # push mask/yn_lo to the idle Pool engine

                def attn_step(h, kt):
                    first = (kt == 0)
                    last = (kt == n_kt - 1)
                    if first:
                        y_ps_l[h] = ps.tile([128, SC], F32, name=f"yps{h}_{c}",
                                            tag="y", bufs=2)
                        sumacc_l[h] = rp.tile([128, SC], F16,
                                              name=f"sum{h}_{c}",
                                              tag="sumacc", bufs=3)
                    y_ps, sumacc = y_ps_l[h], sumacc_l[h]
                    # diagonal blocks: columns j < 128r are fully masked —
                    # compute only the live subrange [j0:].
                    r = kt - 4 * c
                    j0 = 128 * r if r > 0 else 0
                    s_ps = ps.tile([128, SC], F32, name=f"sps{h}_{c}_{kt}",
                                   tag="mm", bufs=5)
                    nc.tensor.matmul(
                        s_ps[:, j0:],
                        KT_sb[kt // 4][:, (kt % 4) * 128:(kt % 4 + 1) * 128],
                        QT[(h, c)][:, j0:], start=True, stop=True)
                    a = rp.tile([128, SC], F16, name=f"A{h}_{c}_{kt}",
                                tag="A", bufs=16)
                    nc.scalar.activation(a[:, j0:], s_ps[:, j0:], ACT.Exp,
                                         bias=b_sb[:, 6:7])
                    if r >= 0:
                        # only the 128-wide diagonal block is partially
                        # masked; columns beyond it are fully unmasked
                        eng = nc.gpsimd if late else nc.vector
                        eng.tensor_mul(a[:, j0:j0 + 128],
                                       a[:, j0:j0 + 128], m_sb[:])
                    nc.tensor.matmul(y_ps[:, j0:], V_sb[kt][:], a[:, j0:],
                                     start=first, stop=last)
                    if first:
                        nc.vector.tensor_copy(sumacc[:], a[:])
                    else:
                        nc.vector.tensor_add(sumacc[:, j0:], sumacc[:, j0:],
                                             a[:, j0:])

                def attn_tail(h):
                    y_ps, sumacc = y_ps_l[h], sumacc_l[h]
                    # ones(1/32) matmul = partition-sum broadcast; rb=32/denom
                    sum_ps = ps.tile([128, SC], F32, name=f"sumps{h}_{c}",
                                     tag="small", bufs=1)
                    nc.tensor.matmul(sum_ps[:], ones_sq[:], sumacc[:],
                                     start=True, stop=True)
                    rb_sb = rp.tile([128, SC], F32, name=f"rb{h}_{c}",
                                    bufs=3, tag="rb")
                    nc.vector.reciprocal(rb_sb[:], sum_ps[:])
                    t_sb = rp.tile([128, SC], F32, name=f"t{h}_{c}",
                                   bufs=3, tag="tn")
                    nc.vector.tensor_mul(t_sb[:], y_ps[:], rb_sb[:])
                    hp, sl = h // 2, h % 2
                    if sl == 0:
                        YnH[c][hp] = rp.tile([128, 2, SC], FP8,
                                             name=f"ynh{c}_{hp}",
                                             tag=f"ynh{hp}", bufs=2)
                        YnL[c][hp] = rp.tile([128, 2, SC], FP8,
                                             name=f"ynl{c}_{hp}",
                                             tag=f"ynl{hp}", bufs=2)
                    nc.vector.tensor_copy(YnH[c][hp][:, sl, :], t_sb[:])
                    eng = nc.gpsimd if late else nc.vector
                    eng.tensor_tensor(
                        YnL[c][hp][:, sl, :], t_sb[:], YnH[c][hp][:, sl, :],
                        mybir.AluOpType.subtract)

                # proportional filler interleave: PE stalls on act's exp
                # backlog otherwise (in-order engine)
                fill_total = sum(t for t, _ in filler)
                fill_done = [0]
                fq = list(filler)
                steps_total = (2 * n_kt if interleave else 4 * n_kt) + 8
                step_i = [0]

                def emit_fill():
                    step_i[0] += 1
                    target = fill_total * step_i[0] / steps_total
                    while fq and fill_done[0] < target:
                        est, fn = fq.pop(0)
                        fn()
                        fill_done[0] += est

                if interleave:
                    for hp in range(0, GQ, 2):
                        for kt in range(n_kt):
                            attn_step(hp, kt)
                            attn_step(hp + 1, kt)
                            emit_fill()
                        attn_tail(hp)
                        attn_tail(hp + 1)
                        emit_fill()
                        if flush_pair:
                            while fq:
                                _, fn = fq.pop(0)
                                fn()
                else:
                    for h in range(GQ):
                        for kt in range(n_kt):
                            attn_step(h, kt)
                            emit_fill()
                        attn_tail(h)
                        emit_fill()
                for est, fn in fq:   # leftovers
                    fn()

            pg0 = proj_groups(0)
            for _, fn in pg0[:4]:
                fn()
            attention(0, pg0[4:], flush_pair=True)
            for _, fn in proj_groups(1):
                fn()
            attention(1, proj_groups(2) + outproj_groups(0))
            attention(2, proj_groups(3) + outproj_groups(1))
            attention(3, outproj_groups(2))
            for _, fn in outproj_groups(3):
                fn()

    _split_multiwait(nc)
    return nc


_SPLIT_N = [0]


def _split_multiwait(nc):
    """Rewrite instructions carrying >1 semaphore wait.

    The walrus build here allows one sync wait per instruction; Tile's
    wait-assignment freely attaches several. Hoist all but the last wait
    onto fresh single-wait NoOps inserted just before the instruction in
    its basic block (engine streams are in-order, so semantics are
    unchanged).
    """
    for f in nc.m.functions:
        for bb in f.blocks:
            il = bb.instructions
            if not any(i.sync_info is not None and len(i.sync_info.on_wait) > 1
                       for i in il):
                continue
            new = []
            for inst in il:
                si = inst.sync_info
                if si is not None and len(si.on_wait) > 1:
                    waits = list(si.on_wait)
                    for w in waits[:-1]:
                        _SPLIT_N[0] += 1
                        new.append(mybir.InstNoOp(
                            name=f"I-waitsplit{_SPLIT_N[0]}",
                            engine=inst.engine,
                            bass_nofuse=True,
                            sync_info=mybir.SyncInfo(on_wait=[w], on_update=[]),
                        ))
                    inst.sync_info = mybir.SyncInfo(
                        on_wait=[waits[-1]], on_update=list(si.on_update))
                new.append(inst)
            bb.instructions = new


E4NP = ml_dtypes.float8_e4m3


def _split8(a):
    hi = a.astype(E4NP)
    lo = (a - hi.astype(np.float32)).astype(E4NP)
    return hi, lo


def _host_prep(x, wq_w, wq_b, wk_w, wk_b, wv_w, wv_b, wo_w, wo_b):
    """Build the 8 per-core input maps, prearranged to SBUF layout."""
    scale = np.float32(1.0 / math.sqrt(HEAD_DIM))

    def to_sbuf_rows(a2d, width):
        # [NT*128, width] -> [128, NT*width] with column blocks = row tiles
        nt = a2d.shape[0] // 128
        return np.ascontiguousarray(
            a2d.reshape(nt, 128, width).transpose(1, 0, 2).reshape(128, nt * width))

    def x_chunks(a8):
        arr = a8.reshape(NT, 128, NSC, SC)             # [dt, p, c, j]
        return [np.ascontiguousarray(
            arr[:, :, c, :].transpose(1, 0, 2).reshape(128, NT * SC))
            for c in range(NSC)]

    xh_b, xl_b = [], []
    for b in range(B):
        h8, l8 = _split8(x[b].T.astype(np.float32))    # [d, s]
        xh_b.append(x_chunks(h8))
        xl_b.append(x_chunks(l8))

    jj = np.arange(128, dtype=np.int32)[None, :]
    pp_ = np.arange(128, dtype=np.int32)[:, None]
    masks = (jj >= pp_).astype(np.float16)

    per_g = []
    for g in range(N_KV):
        wqT = (wq_w[g * EG:(g + 1) * EG, :] * scale).T.astype(np.float32)
        wkT = wk_w[g * HEAD_DIM:(g + 1) * HEAD_DIM, :].T.astype(np.float32)
        wvT = wv_w[g * HEAD_DIM:(g + 1) * HEAD_DIM, :].T.astype(np.float32)
        woT = wo_w[:, g * EG:(g + 1) * EG].T.astype(np.float32)   # [512, f]

        wq_h, wq_l = _split8(wqT * SWQ)       # [d, 512]
        wk_h, wk_l = _split8(wkT * SWK)       # [d, 128]
        wv_h, wv_l = _split8(wvT * SWV)
        wo_h, wo_l = _split8(woT * SWO)       # [512, D]

        # wq layout per dt: [2 heads hi (256) | 2 heads lo (256)] -> wqA
        # holds heads 0,1; wqB heads 2,3
        def wq_pack(h8, l8, h0):
            h3 = h8.reshape(NT, 128, EG)
            l3 = l8.reshape(NT, 128, EG)
            sel = slice(h0 * 256, h0 * 256 + 256)
            packed = np.concatenate([h3[:, :, sel], l3[:, :, sel]], axis=2)
            return np.ascontiguousarray(
                packed.transpose(1, 0, 2).reshape(128, NT * EG))

        wqA = wq_pack(wq_h, wq_l, 0)
        wqB = wq_pack(wq_h, wq_l, 1)

        # wo layout: [hi(et-major) | lo(et-major)], et tiles [128, D]
        wo_cat = np.concatenate(
            [to_sbuf_rows(wo_h, D), to_sbuf_rows(wo_l, D)], axis=1)

        biases = np.empty((128, 7), np.float32)
        biases[:, :GQ] = (wq_b[g * EG:(g + 1) * EG] * scale).reshape(GQ, 128).T
        biases[:, 4] = wk_b[g * HEAD_DIM:(g + 1) * HEAD_DIM]
        biases[:, 5] = wv_b[g * HEAD_DIM:(g + 1) * HEAD_DIM]
        biases[:, 6] = EXP_SHIFT
        per_g.append(dict(
            wqA=wqA, wqB=wqB,
            wkhi=to_sbuf_rows(wk_h, HEAD_DIM),
            wklo=to_sbuf_rows(wk_l, HEAD_DIM),
            wvhi=to_sbuf_rows(wv_h, HEAD_DIM),
            wvlo=to_sbuf_rows(wv_l, HEAD_DIM),
            wo=wo_cat, biases=biases,
        ))

    in_maps = []
    for core in range(8):
        b, g = divmod(core, N_KV)
        m = dict(per_g[g])
        for c in range(NSC):
            m[f"xh{c}"] = xh_b[b][c]
            m[f"xl{c}"] = xl_b[b][c]
        m["masks"] = masks
        in_maps.append(m)
    return in_maps


def kernel(x, wq_w, wq_b, wk_w, wk_b, wv_w, wv_b, wo_w, wo_b, **run_kwargs):
    x = np.asarray(x, dtype=np.float32)
    wq_w = np.asarray(wq_w, dtype=np.float32)
    wq_b = np.asarray(wq_b, dtype=np.float32)
    wk_w = np.asarray(wk_w, dtype=np.float32)
    wk_b = np.asarray(wk_b, dtype=np.float32)
    wv_w = np.asarray(wv_w, dtype=np.float32)
    wv_b = np.asarray(wv_b, dtype=np.float32)
    wo_w = np.asarray(wo_w, dtype=np.float32)
    wo_b = np.asarray(wo_b, dtype=np.float32)

    if "nc" not in _CACHE:
        _CACHE["nc"] = _build_nc()
    nc = _CACHE["nc"]

    in_maps = _host_prep(x, wq_w, wq_b, wk_w, wk_b, wv_w, wv_b, wo_w, wo_b)
    res = run_bass_kernel_spmd(nc, in_maps, core_ids=list(range(8)),
                               **run_kwargs)

    out = np.empty((B, S, D), dtype=np.float32)
    for b in range(B):
        acc = res.results[b * N_KV]["outT"].astype(np.float32)
        for g in range(1, N_KV):
            acc = acc + res.results[b * N_KV + g]["outT"].astype(np.float32)
        out[b] = acc.T * OUT_DESCALE + wo_b[None, :]
    _CACHE["last_res"] = res
    return out
